# revision 3
# baseline (speedup 1.0000x reference)
"""MaxSigmoidAttnBlock Trainium2 kernel.

Reference computation (per batch b):
  g   = (guide @ gl_w.T + gl_b)  -> [n=80, ec=512] -> heads [n, 8, 64]
  emb = BN(conv1x1(x, ec_w))     -> [512, 64, 64]  -> heads [8, 64, h, w]
  aw  = max_n(emb . g) / sqrt(64); aw = sigmoid(aw + bias) * scale  [8, h, w]
  out = BN(conv3x3(x, pj_w)) * aw[head]  -> [512, 64, 64]

Strategy: data-parallel over batch across 8 cores (bs=8 -> 1 batch/core).
BN + attn scales folded into conv weights on host. All matmuls bf16 with
f32 PSUM accumulation. 3x3 conv = 18 shifted matmuls (9 taps x 2 c-tiles)
over a host-padded input. Attention: per e-chunk (2 heads) matmul
emb_chunk^T @ blockdiag(g0,g1) -> [128px, 160] PSUM, free-dim max-reduce,
PE-transpose to [8, px], sigmoid on ACT, then a selector matmul broadcasts
aw back to the 128 channel partitions for the gating multiply.
"""

import os

import numpy as np
import ml_dtypes

import concourse.bass as bass
import concourse.mybir as mybir
import concourse.tile as tile
from concourse import bacc
from concourse.bass_utils import run_bass_kernel_spmd
from concourse.masks import make_identity

EPS = 1e-3
NH = 8
BS = 8
C1 = 256  # input channels
EC = 512  # embed channels
C2 = 512  # output channels
GC = 512  # guide channels
N = 80    # guide tokens
H = W = 64
HC = EC // NH  # 64 head channels
HP = H + 2     # padded
PX = H * W     # 4096 pixels
NCORES = 8

F32 = mybir.dt.float32
BF16 = mybir.dt.bfloat16
BF16_NP = ml_dtypes.bfloat16

_CACHE = {}


def _build_program():
    nc = bacc.Bacc(
        "TRN2",
        target_bir_lowering=False,
        debug=False,
        enable_asserts=False,
        num_devices=NCORES,
    )

    xp_d = nc.dram_tensor("xp", (2, 128, HP * HP), BF16, kind="ExternalInput")
    gt_d = nc.dram_tensor("gt", (4, 128, N), BF16, kind="ExternalInput")
    wec_d = nc.dram_tensor("wec", (2, 128, EC), BF16, kind="ExternalInput")
    wpj_d = nc.dram_tensor("wpj", (2, 128, 9, C2), BF16, kind="ExternalInput")
    glw_d = nc.dram_tensor("glw", (4, 128, EC), BF16, kind="ExternalInput")
    sel_d = nc.dram_tensor("sel", (NH, C2), BF16, kind="ExternalInput")
    bec_d = nc.dram_tensor("bec", (128, 4), F32, kind="ExternalInput")
    bpj_d = nc.dram_tensor("bpj", (128, 4), F32, kind="ExternalInput")
    glb_d = nc.dram_tensor("glb", (128, 4), F32, kind="ExternalInput")
    ab_d = nc.dram_tensor("ab", (NH, 1), F32, kind="ExternalInput")
    out_d = nc.dram_tensor("out", (4, 128, PX), F32, kind="ExternalOutput")

    Ident = mybir.ActivationFunctionType.Identity
    Sigmoid = mybir.ActivationFunctionType.Sigmoid

    with tile.TileContext(nc) as tc:
        with (
            tc.tile_pool(name="const", bufs=1) as const,
            tc.tile_pool(name="work", bufs=3) as work,
            tc.tile_pool(name="psum", bufs=4, space="PSUM") as psum,
        ):
            # ---- persistent SBUF loads ----
            xp = [const.tile([128, HP * HP], BF16, tag=f"xp{i}", name=f"xp{i}") for i in range(2)]
            wec = [const.tile([128, EC], BF16, tag=f"wec{i}", name=f"wec{i}") for i in range(2)]
            wpj = [const.tile([128, 9, C2], BF16, tag=f"wpj{i}", name=f"wpj{i}") for i in range(2)]
            glw = [const.tile([128, EC], BF16, tag=f"glw{i}", name=f"glw{i}") for i in range(4)]
            gt = const.tile([128, 4, N], BF16, tag="gt")
            sel = const.tile([NH, C2], BF16, tag="sel")
            bec = const.tile([128, 4], F32, tag="bec")
            bpj = const.tile([128, 4], F32, tag="bpj")
            glb = const.tile([128, 4], F32, tag="glb")
            ab = const.tile([NH, 1], F32, tag="ab")
            ident = const.tile([128, 128], F32, tag="ident")

            for i in range(2):
                nc.sync.dma_start(xp[i][:], xp_d[i])
                nc.sync.dma_start(wec[i][:], wec_d[i])
                nc.sync.dma_start(wpj[i][:], wpj_d[i])
            for i in range(4):
                nc.sync.dma_start(glw[i][:], glw_d[i])
                nc.sync.dma_start(gt[:, i, :], gt_d[i])
            nc.sync.dma_start(sel[:], sel_d[:])
            nc.sync.dma_start(bec[:], bec_d[:])
            nc.sync.dma_start(bpj[:], bpj_d[:])
            nc.sync.dma_start(glb[:], glb_d[:])
            nc.sync.dma_start(ab[:], ab_d[:])
            make_identity(nc, ident[:])

            # persistent intermediates
            emb = [const.tile([128, PX], BF16, tag=f"emb{k}", name=f"emb{k}") for k in range(4)]
            gpair = [const.tile([128, 2 * N], BF16, tag=f"gpair{k}", name=f"gpair{k}") for k in range(4)]
            awcol = const.tile([128, 32, NH], F32, tag="awcol")
            awt = const.tile([NH, PX], BF16, tag="awt")

            xv = [xp[i][:].rearrange("p (h w) -> p h w", h=HP) for i in range(2)]

            # ---- Phase G: guide projection -> block-diag gpair per e-chunk ----
            for k in range(4):
                pg = psum.tile([128, 2 * N], F32, tag="attn", bufs=3)
                for t in range(4):
                    nc.tensor.matmul(
                        pg[:, :N],
                        glw[t][:, k * 128:(k + 1) * 128],
                        gt[:, t, :],
                        start=(t == 0),
                        stop=(t == 3),
                    )
                nc.vector.memset(gpair[k][:], 0.0)
                nc.scalar.activation(
                    gpair[k][0:64, 0:N], pg[0:64, :N], Ident,
                    bias=glb[0:64, k:k + 1],
                )
                nc.scalar.activation(
                    gpair[k][64:128, N:2 * N], pg[64:128, :N], Ident,
                    bias=glb[64:128, k:k + 1],
                )

            # ---- Phase E: conv1x1 + BN -> emb (bf16) ----
            for k in range(4):
                for j in range(8):
                    pe = psum.tile([128, 512], F32, tag="mm")
                    for ct in range(2):
                        nc.tensor.matmul(
                            pe[:],
                            wec[ct][:, k * 128:(k + 1) * 128],
                            xv[ct][:, j * 8 + 1:j * 8 + 9, 1:65],
                            start=(ct == 0),
                            stop=(ct == 1),
                        )
                    nc.scalar.activation(
                        emb[k][:, j * 512:(j + 1) * 512], pe[:], Ident,
                        bias=bec[:, k:k + 1],
                    )

            # ---- Phase A: attention logits + max over tokens ----
            for k in range(4):
                for i in range(32):
                    pa = psum.tile([128, 2 * N], F32, tag="attn", bufs=3)
                    nc.tensor.matmul(
                        pa[:],
                        emb[k][:, i * 128:(i + 1) * 128],
                        gpair[k][:],
                        start=True,
                        stop=True,
                    )
                    nc.vector.tensor_reduce(
                        awcol[:, i, 2 * k:2 * k + 2],
                        pa[:].rearrange("p (t n) -> p t n", t=2),
                        axis=mybir.AxisListType.X,
                        op=mybir.AluOpType.max,
                    )

            # ---- Phase T: transpose [128px, 8] -> [8, 128px], sigmoid ----
            for i in range(32):
                pt = psum.tile([128, 128], F32, tag="tp", bufs=1)
                nc.tensor.transpose(pt[0:NH, :], awcol[:, i, :], ident[:])
                nc.scalar.activation(
                    awt[:, i * 128:(i + 1) * 128], pt[0:NH, :], Sigmoid,
                    bias=ab[:, 0:1],
                )

            # ---- Phase P: conv3x3 + BN, broadcast aw, gate, store ----
            for k in range(4):
                for j in range(8):
                    pc = psum.tile([128, 512], F32, tag="mm")
                    idx = 0
                    for ky in range(3):
                        for kx in range(3):
                            for ct in range(2):
                                nc.tensor.matmul(
                                    pc[:],
                                    wpj[ct][:, ky * 3 + kx, k * 128:(k + 1) * 128],
                                    xv[ct][:, j * 8 + ky:j * 8 + 8 + ky, kx:kx + 64],
                                    start=(idx == 0),
                                    stop=(idx == 17),
                                )
                                idx += 1
                    prep = psum.tile([128, 512], F32, tag="mm")
                    nc.tensor.matmul(
                        prep[:],
                        sel[:, k * 128:(k + 1) * 128],
                        awt[:, j * 512:(j + 1) * 512],
                        start=True,
                        stop=True,
                    )
                    c3 = work.tile([128, 512], F32, tag="c3")
                    nc.scalar.activation(
                        c3[:], pc[:], Ident, bias=bpj[:, k:k + 1],
                    )
                    osb = work.tile([128, 512], F32, tag="osb")
                    nc.vector.tensor_mul(osb[:], c3[:], prep[:])
                    nc.sync.dma_start(out_d[k, :, j * 512:(j + 1) * 512], osb[:])

    nc.compile()
    return nc


def _prep_inputs(inputs):
    """Host-side layout prep + BN folding. Returns per-core in_maps."""
    f = np.float32
    x = np.asarray(inputs["x"], f)
    guide = np.asarray(inputs["guide"], f)
    ec_w = np.asarray(inputs["ec_w"], f)
    ec_g = np.asarray(inputs["ec_g"], f)
    ec_b = np.asarray(inputs["ec_b"], f)
    ec_m = np.asarray(inputs["ec_m"], f)
    ec_v = np.asarray(inputs["ec_v"], f)
    gl_w = np.asarray(inputs["gl_w"], f)
    gl_b = np.asarray(inputs["gl_b"], f)
    attn_bias = np.asarray(inputs["attn_bias"], f)
    attn_scale = np.asarray(inputs["attn_scale"], f).reshape(NH)
    pj_w = np.asarray(inputs["pj_w"], f)
    pj_g = np.asarray(inputs["pj_g"], f)
    pj_b = np.asarray(inputs["pj_b"], f)
    pj_m = np.asarray(inputs["pj_m"], f)
    pj_v = np.asarray(inputs["pj_v"], f)

    s_ec = ec_g / np.sqrt(ec_v + EPS)
    w_ec = ec_w[:, :, 0, 0] * s_ec[:, None]            # [EC, C1]
    b_ec = ec_b - ec_m * s_ec                          # [EC]

    scale_ch = np.repeat(attn_scale, HC)               # [C2]
    s_pj = pj_g / np.sqrt(pj_v + EPS)
    w_pj = pj_w * (s_pj * scale_ch)[:, None, None, None]
    b_pj = (pj_b - pj_m * s_pj) * scale_ch

    inv = 1.0 / np.sqrt(HC)
    glw_eff = gl_w * inv
    glb_eff = gl_b * inv

    bf = BF16_NP
    x_pad = np.pad(x, ((0, 0), (0, 0), (1, 1), (1, 1)))
    xp = np.ascontiguousarray(
        x_pad.reshape(BS, 2, 128, HP * HP).astype(bf))
    gt = np.ascontiguousarray(
        guide.transpose(0, 2, 1).reshape(BS, 4, 128, N).astype(bf))
    wec_t = np.ascontiguousarray(w_ec.T.reshape(2, 128, EC).astype(bf))
    wpj_t = np.ascontiguousarray(
        w_pj.transpose(1, 2, 3, 0).reshape(2, 128, 9, C2).astype(bf))
    glw_t = np.ascontiguousarray(glw_eff.T.reshape(4, 128, EC).astype(bf))
    sel = np.zeros((NH, C2), bf)
    sel[np.arange(C2) // HC, np.arange(C2)] = 1
    bec_t = np.ascontiguousarray(b_ec.reshape(4, 128).T)
    bpj_t = np.ascontiguousarray(b_pj.reshape(4, 128).T)
    glb_t = np.ascontiguousarray(glb_eff.reshape(4, 128).T)
    ab = np.ascontiguousarray(attn_bias.reshape(NH, 1))

    shared = {
        "wec": wec_t, "wpj": wpj_t, "glw": glw_t, "sel": sel,
        "bec": bec_t, "bpj": bpj_t, "glb": glb_t, "ab": ab,
    }
    return [dict(shared, xp=xp[b], gt=gt[b]) for b in range(BS)]


def _run(inputs, trace=False, **kw):
    if "nc" not in _CACHE:
        _CACHE["nc"] = _build_program()
    nc = _CACHE["nc"]
    in_maps = _prep_inputs(inputs)
    res = run_bass_kernel_spmd(
        nc, in_maps, core_ids=list(range(NCORES)), trace=trace, **kw)
    out = np.stack(
        [res.results[b]["out"].reshape(C2, H, W) for b in range(BS)])
    return out.astype(np.float32), res


def kernel(**inputs):
    out, _ = _run(inputs, trace=False)
    return out


# revision 4
# speedup vs baseline: 90.8871x; 90.8871x over previous
"""MaxSigmoidAttnBlock Trainium2 kernel.

Reference computation (per batch b):
  g   = (guide @ gl_w.T + gl_b)  -> [n=80, ec=512] -> heads [n, 8, 64]
  emb = BN(conv1x1(x, ec_w))     -> [512, 64, 64]  -> heads [8, 64, h, w]
  aw  = max_n(emb . g) / sqrt(64); aw = sigmoid(aw + bias) * scale  [8, h, w]
  out = BN(conv3x3(x, pj_w)) * aw[head]  -> [512, 64, 64]

Strategy: data-parallel over batch across 8 cores (bs=8 -> 1 batch/core).
BN + attn scales folded into conv weights on host. All matmuls bf16 with
f32 PSUM accumulation. 3x3 conv = 18 shifted matmuls (9 taps x 2 c-tiles)
over a host-padded input. Attention: per e-chunk (2 heads) matmul
emb_chunk^T @ blockdiag(g0,g1) -> [128px, 160] PSUM, free-dim max-reduce,
PE-transpose to [8, px], sigmoid on ACT, then a selector matmul broadcasts
aw back to the 128 channel partitions for the gating multiply.

`rep` (benchmark only): re-issues the compute phases `rep` times inside one
NEFF so (T(rep=R) - T(rep=1)) / (R-1) isolates steady-state device time
from the axon dispatch overhead.
"""

import numpy as np
import ml_dtypes

import concourse.bass as bass
import concourse.mybir as mybir
import concourse.tile as tile
from concourse import bacc
from concourse.bass_utils import run_bass_kernel_spmd
from concourse.masks import make_identity

EPS = 1e-3
NH = 8
BS = 8
C1 = 256  # input channels
EC = 512  # embed channels
C2 = 512  # output channels
GC = 512  # guide channels
N = 80    # guide tokens
H = W = 64
HC = EC // NH  # 64 head channels
HP = H + 2     # padded
PX = H * W     # 4096 pixels
NCORES = 8

F32 = mybir.dt.float32
BF16 = mybir.dt.bfloat16
BF16_NP = ml_dtypes.bfloat16

_CACHE = {}


def _build_program(rep=1):
    nc = bacc.Bacc(
        "TRN2",
        target_bir_lowering=False,
        debug=False,
        enable_asserts=False,
        num_devices=NCORES,
    )

    xp_d = nc.dram_tensor("xp", (2, 128, HP * HP), BF16, kind="ExternalInput")
    gt_d = nc.dram_tensor("gt", (4, 128, N), BF16, kind="ExternalInput")
    wec_d = nc.dram_tensor("wec", (2, 128, EC), BF16, kind="ExternalInput")
    wpj_d = nc.dram_tensor("wpj", (2, 128, 9, C2), BF16, kind="ExternalInput")
    glw_d = nc.dram_tensor("glw", (4, 128, EC), BF16, kind="ExternalInput")
    sel_d = nc.dram_tensor("sel", (NH, C2), BF16, kind="ExternalInput")
    bec_d = nc.dram_tensor("bec", (128, 4), F32, kind="ExternalInput")
    bpj_d = nc.dram_tensor("bpj", (128, 4), F32, kind="ExternalInput")
    glb_d = nc.dram_tensor("glb", (128, 4), F32, kind="ExternalInput")
    ab_d = nc.dram_tensor("ab", (NH, 1), F32, kind="ExternalInput")
    out_d = nc.dram_tensor("out", (4, 128, PX), F32, kind="ExternalOutput")

    Ident = mybir.ActivationFunctionType.Identity
    Sigmoid = mybir.ActivationFunctionType.Sigmoid

    with tile.TileContext(nc) as tc:
        with (
            tc.tile_pool(name="const", bufs=1) as const,
            tc.tile_pool(name="inter", bufs=1) as inter,
            tc.tile_pool(name="work", bufs=3) as work,
            tc.tile_pool(name="psum", bufs=4, space="PSUM") as psum,
        ):
            # ---- persistent SBUF loads ----
            xp = [const.tile([128, HP * HP], BF16, tag=f"xp{i}", name=f"xp{i}")
                  for i in range(2)]
            wec = [const.tile([128, EC], BF16, tag=f"wec{i}", name=f"wec{i}")
                   for i in range(2)]
            wpj = [const.tile([128, 9, C2], BF16, tag=f"wpj{i}", name=f"wpj{i}")
                   for i in range(2)]
            glw = [const.tile([128, EC], BF16, tag=f"glw{i}", name=f"glw{i}")
                   for i in range(4)]
            gt = const.tile([128, 4, N], BF16, tag="gt")
            sel = const.tile([NH, C2], BF16, tag="sel")
            bec = const.tile([128, 4], F32, tag="bec")
            bpj = const.tile([128, 4], F32, tag="bpj")
            glb = const.tile([128, 4], F32, tag="glb")
            ab = const.tile([NH, 1], F32, tag="ab")
            ident = const.tile([128, 128], F32, tag="ident")

            for i in range(2):
                nc.sync.dma_start(xp[i][:], xp_d[i])
                nc.sync.dma_start(wec[i][:], wec_d[i])
                nc.sync.dma_start(wpj[i][:], wpj_d[i])
            for i in range(4):
                nc.sync.dma_start(glw[i][:], glw_d[i])
                nc.sync.dma_start(gt[:, i, :], gt_d[i])
            nc.sync.dma_start(sel[:], sel_d[:])
            nc.sync.dma_start(bec[:], bec_d[:])
            nc.sync.dma_start(bpj[:], bpj_d[:])
            nc.sync.dma_start(glb[:], glb_d[:])
            nc.sync.dma_start(ab[:], ab_d[:])
            make_identity(nc, ident[:])

            xv = [xp[i][:].rearrange("p (h w) -> p h w", h=HP) for i in range(2)]

            for _ in range(rep):
                _body(nc, inter, work, psum, xv, wec, wpj, glw, gt, sel,
                      bec, bpj, glb, ab, ident, out_d, Ident, Sigmoid)

    nc.compile()
    return nc


def _body(nc, inter, work, psum, xv, wec, wpj, glw, gt, sel,
          bec, bpj, glb, ab, ident, out_d, Ident, Sigmoid):
    emb = [inter.tile([128, PX], BF16, tag=f"emb{k}", name=f"emb{k}")
           for k in range(4)]
    gpair = [inter.tile([128, 2 * N], BF16, tag=f"gpair{k}", name=f"gpair{k}")
             for k in range(4)]
    awcol = inter.tile([128, 32, NH], F32, tag="awcol")
    awt = inter.tile([NH, PX], BF16, tag="awt")

    # ---- Phase G: guide projection -> block-diag gpair per e-chunk ----
    for k in range(4):
        pg = psum.tile([128, 2 * N], F32, tag="attn", bufs=3)
        for t in range(4):
            nc.tensor.matmul(
                pg[:, :N],
                glw[t][:, k * 128:(k + 1) * 128],
                gt[:, t, :],
                start=(t == 0),
                stop=(t == 3),
            )
        nc.vector.memset(gpair[k][:], 0.0)
        nc.scalar.activation(
            gpair[k][0:64, 0:N], pg[0:64, :N], Ident,
            bias=glb[0:64, k:k + 1],
        )
        nc.scalar.activation(
            gpair[k][64:128, N:2 * N], pg[64:128, :N], Ident,
            bias=glb[64:128, k:k + 1],
        )

    # ---- Phase E: conv1x1 + BN -> emb (bf16) ----
    for k in range(4):
        for j in range(8):
            pe = psum.tile([128, 512], F32, tag="mm")
            for ct in range(2):
                nc.tensor.matmul(
                    pe[:],
                    wec[ct][:, k * 128:(k + 1) * 128],
                    xv[ct][:, j * 8 + 1:j * 8 + 9, 1:65],
                    start=(ct == 0),
                    stop=(ct == 1),
                )
            nc.scalar.activation(
                emb[k][:, j * 512:(j + 1) * 512], pe[:], Ident,
                bias=bec[:, k:k + 1],
            )

    # ---- Phase A: attention logits + max over tokens ----
    for k in range(4):
        for i in range(32):
            pa = psum.tile([128, 2 * N], F32, tag="attn", bufs=3)
            nc.tensor.matmul(
                pa[:],
                emb[k][:, i * 128:(i + 1) * 128],
                gpair[k][:],
                start=True,
                stop=True,
            )
            nc.vector.tensor_reduce(
                awcol[:, i, 2 * k:2 * k + 2],
                pa[:].rearrange("p (t n) -> p t n", t=2),
                axis=mybir.AxisListType.X,
                op=mybir.AluOpType.max,
            )

    # ---- Phase T: transpose [128px, 8] -> [8, 128px], sigmoid ----
    for i in range(32):
        pt = psum.tile([128, 128], F32, tag="tp", bufs=1)
        nc.tensor.transpose(pt[0:NH, :], awcol[:, i, :], ident[:])
        nc.scalar.activation(
            awt[:, i * 128:(i + 1) * 128], pt[0:NH, :], Sigmoid,
            bias=ab[:, 0:1],
        )

    # ---- Phase P: conv3x3 + BN, broadcast aw, gate, store ----
    for k in range(4):
        for j in range(8):
            pc = psum.tile([128, 512], F32, tag="mm")
            idx = 0
            for ky in range(3):
                for kx in range(3):
                    for ct in range(2):
                        nc.tensor.matmul(
                            pc[:],
                            wpj[ct][:, ky * 3 + kx, k * 128:(k + 1) * 128],
                            xv[ct][:, j * 8 + ky:j * 8 + 8 + ky, kx:kx + 64],
                            start=(idx == 0),
                            stop=(idx == 17),
                        )
                        idx += 1
            prep = psum.tile([128, 512], F32, tag="mm")
            nc.tensor.matmul(
                prep[:],
                sel[:, k * 128:(k + 1) * 128],
                awt[:, j * 512:(j + 1) * 512],
                start=True,
                stop=True,
            )
            c3 = work.tile([128, 512], F32, tag="c3")
            nc.scalar.activation(
                c3[:], pc[:], Ident, bias=bpj[:, k:k + 1],
            )
            osb = work.tile([128, 512], F32, tag="osb")
            nc.vector.tensor_mul(osb[:], c3[:], prep[:])
            nc.sync.dma_start(out_d[k, :, j * 512:(j + 1) * 512], osb[:])


def _prep_inputs(inputs):
    """Host-side layout prep + BN folding. Returns per-core in_maps."""
    f = np.float32
    x = np.asarray(inputs["x"], f)
    guide = np.asarray(inputs["guide"], f)
    ec_w = np.asarray(inputs["ec_w"], f)
    ec_g = np.asarray(inputs["ec_g"], f)
    ec_b = np.asarray(inputs["ec_b"], f)
    ec_m = np.asarray(inputs["ec_m"], f)
    ec_v = np.asarray(inputs["ec_v"], f)
    gl_w = np.asarray(inputs["gl_w"], f)
    gl_b = np.asarray(inputs["gl_b"], f)
    attn_bias = np.asarray(inputs["attn_bias"], f)
    attn_scale = np.asarray(inputs["attn_scale"], f).reshape(NH)
    pj_w = np.asarray(inputs["pj_w"], f)
    pj_g = np.asarray(inputs["pj_g"], f)
    pj_b = np.asarray(inputs["pj_b"], f)
    pj_m = np.asarray(inputs["pj_m"], f)
    pj_v = np.asarray(inputs["pj_v"], f)

    s_ec = ec_g / np.sqrt(ec_v + EPS)
    w_ec = ec_w[:, :, 0, 0] * s_ec[:, None]            # [EC, C1]
    b_ec = ec_b - ec_m * s_ec                          # [EC]

    scale_ch = np.repeat(attn_scale, HC)               # [C2]
    s_pj = pj_g / np.sqrt(pj_v + EPS)
    w_pj = pj_w * (s_pj * scale_ch)[:, None, None, None]
    b_pj = (pj_b - pj_m * s_pj) * scale_ch

    inv = 1.0 / np.sqrt(HC)
    glw_eff = gl_w * inv
    glb_eff = gl_b * inv

    bf = BF16_NP
    x_pad = np.pad(x, ((0, 0), (0, 0), (1, 1), (1, 1)))
    xp = np.ascontiguousarray(
        x_pad.reshape(BS, 2, 128, HP * HP).astype(bf))
    gt = np.ascontiguousarray(
        guide.transpose(0, 2, 1).reshape(BS, 4, 128, N).astype(bf))
    wec_t = np.ascontiguousarray(w_ec.T.reshape(2, 128, EC).astype(bf))
    wpj_t = np.ascontiguousarray(
        w_pj.transpose(1, 2, 3, 0).reshape(2, 128, 9, C2).astype(bf))
    glw_t = np.ascontiguousarray(glw_eff.T.reshape(4, 128, EC).astype(bf))
    sel = np.zeros((NH, C2), bf)
    sel[np.arange(C2) // HC, np.arange(C2)] = 1
    bec_t = np.ascontiguousarray(b_ec.reshape(4, 128).T)
    bpj_t = np.ascontiguousarray(b_pj.reshape(4, 128).T)
    glb_t = np.ascontiguousarray(glb_eff.reshape(4, 128).T)
    ab = np.ascontiguousarray(attn_bias.reshape(NH, 1))

    shared = {
        "wec": wec_t, "wpj": wpj_t, "glw": glw_t, "sel": sel,
        "bec": bec_t, "bpj": bpj_t, "glb": glb_t, "ab": ab,
    }
    return [dict(shared, xp=xp[b], gt=gt[b]) for b in range(BS)]


def _run(inputs, trace=False, **kw):
    if "nc" not in _CACHE:
        _CACHE["nc"] = _build_program()
    nc = _CACHE["nc"]
    in_maps = _prep_inputs(inputs)
    res = run_bass_kernel_spmd(
        nc, in_maps, core_ids=list(range(NCORES)), trace=trace, **kw)
    out = np.stack(
        [res.results[b]["out"].reshape(C2, H, W) for b in range(BS)])
    return out.astype(np.float32), res


def kernel(**inputs):
    out, _ = _run(inputs, trace=False)
    return out
